# revision 1
# baseline (speedup 1.0000x reference)
"""Trainium2 Bass kernel: MEGNet GlobalModel (graph aggregation + 3-layer MLP w/ BatchNorm).

Strategy (graph-parallel over 8 NeuronCores):
  - 2048 graphs -> 16 windows of 128 graphs; core c owns windows 2c, 2c+1.
  - Host folds the two chained scatter_means into ONE weighted segment-sum:
        u_e[g] = sum_{e: batch[src_e]=g} edge_attr[e] * (1/max(deg[src_e],1)) * (1/max(cnt[g],1))
        u_v[g] = sum_{n: batch[n]=g}    x[n]        * (1/max(cnt[g],1))
    Host sorts edges by graph id and partitions edges/nodes by window (index-only
    metadata + permutation/layout); the reductions and the MLP run on device.
  - Device: per 128-row tile, a selection matrix sel[i, j] = (gid_i == j) * w_i is
    matmul'ed against the data tile, accumulating per-graph sums in PSUM (sorted
    segment reduction). Because rows are sorted, each tile only touches a 64-wide
    aligned graph slot (host-computed, unioned across cores so the SPMD program is
    shared); sel is built for a whole 16-tile chunk with two broadcast f16
    tensor_tensor ops against host-rebased gids. Rare tiles crossing a 64-graph
    line get a per-tile fixup op into the other slot.
  - Streaming data is fp16 (PSUM accumulation fp32); DRAM is pre-tiled chunk-major
    on host so every DMA descriptor moves 8KB contiguous per partition.
  - Per-core comb^T = [u_e^T; u_v^T; u^T] is AllGather'ed per 128-graph window
    (first gather overlaps the second window's streaming); every core then runs
    the replicated fp32 MLP on all 2048 graphs in [feature, graph] layout
    (BatchNorm batch stats reduce along the free axis) and writes out^T.
"""

import sys

sys.path.insert(0, "/opt/trn_rl_repo")

import numpy as np

from concourse import bacc, bass, bass_utils, mybir, tile
from concourse.masks import make_identity

F32 = mybir.dt.float32
F16 = mybir.dt.float16
P = 128
EPS = 1e-5
NCORES = 8
WPC = 2  # graph windows per core
CH = 16  # [128, D] tiles per DMA chunk
SW = 64  # graph slot width a non-initial tile may touch
ALU = mybir.AluOpType
ACTF = mybir.ActivationFunctionType
AX = mybir.AxisListType

_prog_cache: dict = {}


def _ceil_to(a: int, m: int) -> int:
    return -(-a // m) * m


# ---------------------------------------------------------------- device program


def _emit(nc, tc, cfg, ap):
    D, NG, EW, XW = cfg["D"], cfg["NG"], cfg["EW"], cfg["XW"]
    nt_e, nt_x = EW // P, XW // P
    DJ = D // P  # feature tiles per 128 partitions (2)
    K1 = 3 * D // P  # k-tiles of layer 1 (6)
    GPC = NG // NCORES  # graphs per core (256)
    WIN = NG // (NCORES * WPC)  # graphs per window (128)
    assert WIN == P and D % P == 0 and NG % 512 == 0

    with (
        tc.tile_pool(name="const", bufs=1) as cpool,
        tc.tile_pool(name="tables", bufs=1) as tpool,
        tc.tile_pool(name="data", bufs=4) as dpool,
        tc.tile_pool(name="eq", bufs=3) as qpool,
        tc.tile_pool(name="sel", bufs=4) as spool,
        tc.tile_pool(name="evac", bufs=2) as epool,
        tc.tile_pool(name="psum", bufs=2, space="PSUM") as ppool,
        tc.tile_pool(name="mlp", bufs=1) as mpool,
        tc.tile_pool(name="stats", bufs=2) as stpool,
        tc.tile_pool(name="dram", bufs=1, space="DRAM") as drpool,
    ):
        # --- constants
        ident = cpool.tile([P, P], F32)
        make_identity(nc, ident[:])
        iota_i = cpool.tile([P, P], mybir.dt.int32)
        nc.gpsimd.iota(iota_i[:], pattern=[[1, P]], base=0, channel_multiplier=0)
        iota16 = cpool.tile([P, P], F16)
        nc.vector.tensor_copy(iota16[:], iota_i[:])
        eps_sb = cpool.tile([P, 1], F32)
        nc.vector.memset(eps_sb[:], EPS)
        iota3 = iota16[:, 0:SW].rearrange("p (o f) -> p o f", o=1)

        # --- per-row tables: rebased gid + weight, f16 (chunk ops) + f32 (scalar ops)
        def table(name, cols, dt):
            t = tpool.tile([P, cols], dt, name=name)
            nc.sync.dma_start(t[:], ap[name][:, :])
            return t

        eg16 = table("eg16", WPC * nt_e, F16)
        ew16 = table("ew16", WPC * nt_e, F16)
        eg32 = table("eg32", WPC * nt_e, F32)
        ew32 = table("ew32", WPC * nt_e, F32)
        xg16 = table("xg16", WPC * nt_x, F16)
        xw16 = table("xw16", WPC * nt_x, F16)
        xg32 = table("xg32", WPC * nt_x, F32)
        xw32 = table("xw32", WPC * nt_x, F32)

        # --- MLP params, prefetched up front
        gsbw = [
            mpool.tile([P, NCORES, K1, WIN], F16, name=f"gsb{w}") for w in range(WPC)
        ]
        w0_sb = mpool.tile([P, K1, D], F16)
        nc.sync.dma_start(w0_sb[:], ap["w0t"][:, :].rearrange("(a p) f -> p a f", p=P))
        w1_sb = mpool.tile([P, DJ, D], F16)
        nc.sync.dma_start(w1_sb[:], ap["w1t"][:, :].rearrange("(a p) f -> p a f", p=P))
        w2_sb = mpool.tile([P, DJ, D], F16)
        nc.sync.dma_start(w2_sb[:], ap["w2t"][:, :].rearrange("(a p) f -> p a f", p=P))
        par_sb = mpool.tile([P, DJ, 9], F32)
        nc.sync.dma_start(par_sb[:], ap["par"][:, :].rearrange("(a p) c -> p a c", p=P))

        # --- DRAM bounce buffers, one collective per window
        combs, gaths = [], []
        for w in range(WPC):
            cb = drpool.tile([3 * D, WIN], F16, name=f"comb{w}")
            gt = drpool.tile(
                [NCORES, 3 * D, WIN], F16, addr_space="Shared", name=f"gath{w}"
            )
            nc.sync.dma_start(cb[2 * D : 3 * D, :], ap["ut"][:, w * WIN : (w + 1) * WIN])
            combs.append(cb)
            gaths.append(gt)

        # --- one window of weighted segment-sum: acc[g, :] += sel.T @ rows
        def seg_window(data_ap, g16, w16, g32, w32, nt, win, comb_dst, row0, meta):
            bases, fixups = meta
            acc = ppool.tile([P, D], F32, tag="acc")
            nchunks = nt // CH
            for c in range(nchunks):
                r0 = (win * nchunks + c) * P
                chunk = dpool.tile([P, CH, D], F16, tag="data")
                # window 1 streams on the sync ring only: the scalar ring holds
                # the gather-load that waits on the window-0 collective
                eng = nc.scalar if (win == 0 and c % 2 == 1) else nc.sync
                eng.dma_start(chunk[:], data_ap[r0 : r0 + P, :])
                cl, cr = win * nt + c * CH, win * nt + (c + 1) * CH
                eq = qpool.tile([P, CH, SW], F16, tag="eq")
                nc.vector.tensor_tensor(
                    out=eq[:],
                    in0=iota3.to_broadcast([P, CH, SW]),
                    in1=g16[:, cl:cr].rearrange("p (c o) -> p c o", o=1).to_broadcast(
                        [P, CH, SW]
                    ),
                    op=ALU.is_equal,
                )
                selc = spool.tile([P, CH, SW], F16, tag="sel")
                nc.vector.tensor_tensor(
                    out=selc[:],
                    in0=eq[:],
                    in1=w16[:, cl:cr].rearrange("p (c o) -> p c o", o=1).to_broadcast(
                        [P, CH, SW]
                    ),
                    op=ALU.mult,
                )
                for s in range(CH):
                    t = c * CH + s
                    col = win * nt + t
                    rows = chunk[:, s, :]
                    if t == 0:
                        sel0 = spool.tile([P, P], F16, tag="sel0")
                        nc.vector.tensor_scalar(
                            sel0[:],
                            iota16[:, 0:P],
                            scalar1=g32[:, col : col + 1],
                            scalar2=w32[:, col : col + 1],
                            op0=ALU.is_equal,
                            op1=ALU.mult,
                        )
                        nc.tensor.matmul(
                            acc[:], lhsT=sel0[:], rhs=rows,
                            start=True, stop=False, skip_group_check=True,
                        )
                        continue
                    b = bases[t]
                    nc.tensor.matmul(
                        acc[b : b + SW, :], lhsT=selc[:, s, :], rhs=rows,
                        start=False, stop=(t == nt - 1), skip_group_check=True,
                    )
                    if t in fixups:
                        # tile crosses the 64-graph line: cover the upper slot
                        sf = spool.tile([P, SW], F16, tag="sfix")
                        nc.vector.tensor_scalar(
                            sf[:],
                            iota16[:, SW : 2 * SW],
                            scalar1=g32[:, col : col + 1],
                            scalar2=w32[:, col : col + 1],
                            op0=ALU.is_equal,
                            op1=ALU.mult,
                        )
                        nc.tensor.matmul(
                            acc[SW : 2 * SW, :], lhsT=sf[:], rhs=rows,
                            start=False, stop=False, skip_group_check=True,
                        )
            # evacuate: acc is [graph, feat]; transpose 128x128 blocks into comb^T
            acc_sb = epool.tile([P, D], F32, tag="acc_sb")
            nc.scalar.copy(acc_sb[:], acc[:])
            for fh in range(DJ):
                tp = ppool.tile([P, P], F32, tag="mm")
                nc.tensor.transpose(tp[:], acc_sb[:, fh * P : (fh + 1) * P], ident[:])
                tsb = epool.tile([P, P], F16, tag="tsb")
                nc.scalar.copy(tsb[:], tp[:])
                nc.sync.dma_start(
                    comb_dst[row0 + fh * P : row0 + (fh + 1) * P, :], tsb[:]
                )

        for win in range(WPC):
            seg_window(
                ap["xa"], xg16, xw16, xg32, xw32, nt_x, win, combs[win], D,
                cfg["xmeta"][win],
            )
            seg_window(
                ap["ea"], eg16, ew16, eg32, ew32, nt_e, win, combs[win], 0,
                cfg["emeta"][win],
            )
            nc.gpsimd.collective_compute(
                "AllGather",
                ALU.bypass,
                replica_groups=[list(range(NCORES))],
                ins=[combs[win].opt()],
                outs=[gaths[win].opt()],
            )
            # scalar HWDGE ring: idle during window-1 streaming, so waiting on
            # the collective here blocks nothing (and HWDGE descriptor
            # generation does not take the GpSimd/DVE shared SBUF port)
            nc.scalar.dma_start(
                gsbw[win][:],
                gaths[win][:, :, :].rearrange("r (a p) g -> p r a g", p=P),
            )

        # --- replicated MLP over all NG graphs, [feature, graph] layout
        h0 = mpool.tile([P, DJ, NG], F16, name="h0")
        h1 = mpool.tile([P, DJ, NG], F16, name="h1")
        hout = mpool.tile([P, DJ, NG], F32, name="hout")

        def mlp_layer(L, w_sb, nk, mm_rhs, act_out, h_out):
            # 2 activation chunks of NG/2 columns per feature tile; each act
            # chunk is filled by ACW/512 accumulating matmul groups
            ACW = NG // 2
            SUB = ACW // 512
            s_sum = stpool.tile([P, DJ, 2], F32, tag="ssum")
            s_sq = stpool.tile([P, DJ, 2], F32, tag="ssq")
            for jt in range(DJ):
                for ac in range(2):
                    ps = ppool.tile([P, SUB, 512], F32, tag="mm")
                    for sub in range(SUB):
                        for kk in range(nk):
                            nc.tensor.matmul(
                                ps[:, sub, :],
                                lhsT=w_sb[:, kk, jt * P : (jt + 1) * P],
                                rhs=mm_rhs(kk, ac, sub),
                                start=(kk == 0),
                                stop=(kk == nk - 1),
                            )
                    hsl = act_out(h_out, jt, ac)
                    nc.scalar.activation(
                        hsl,
                        ps[:].rearrange("p a b -> p (a b)"),
                        ACTF.Relu,
                        bias=par_sb[:, jt, L : L + 1],
                        scale=1.0,
                        accum_out=s_sum[:, jt, ac : ac + 1],
                    )
                    scr = stpool.tile([P, ACW], F16, tag="scr")
                    nc.scalar.activation(
                        scr[:], hsl, ACTF.Square, scale=1.0,
                        accum_out=s_sq[:, jt, ac : ac + 1],
                    )
            # BatchNorm stats for both feature tiles in one [P, DJ] chain
            tsum = stpool.tile([P, DJ], F32, tag="tsum")
            nc.vector.tensor_reduce(tsum[:], s_sum[:], axis=AX.X, op=ALU.add)
            tsq = stpool.tile([P, DJ], F32, tag="tsq")
            nc.vector.tensor_reduce(tsq[:], s_sq[:], axis=AX.X, op=ALU.add)
            mean = stpool.tile([P, DJ], F32, tag="mean")
            nc.vector.tensor_scalar_mul(mean[:], tsum[:], 1.0 / NG)
            esq = stpool.tile([P, DJ], F32, tag="esq")
            nc.vector.tensor_scalar_mul(esq[:], tsq[:], 1.0 / NG)
            m2 = stpool.tile([P, DJ], F32, tag="m2")
            nc.vector.tensor_tensor(out=m2[:], in0=mean[:], in1=mean[:], op=ALU.mult)
            var = stpool.tile([P, DJ], F32, tag="var")
            nc.vector.tensor_tensor(out=var[:], in0=esq[:], in1=m2[:], op=ALU.subtract)
            std = stpool.tile([P, DJ], F32, tag="std")
            nc.scalar.activation(std[:], var[:], ACTF.Sqrt, bias=eps_sb[:], scale=1.0)
            rstd = stpool.tile([P, DJ], F32, tag="rstd")
            nc.vector.reciprocal(rstd[:], std[:])
            scl = stpool.tile([P, DJ], F32, tag="scl")
            nc.vector.tensor_tensor(
                out=scl[:], in0=rstd[:], in1=par_sb[:, :, 3 + L], op=ALU.mult
            )
            mscl = stpool.tile([P, DJ], F32, tag="mscl")
            nc.vector.tensor_tensor(out=mscl[:], in0=mean[:], in1=scl[:], op=ALU.mult)
            bv = stpool.tile([P, DJ], F32, tag="bv")
            nc.vector.tensor_tensor(
                out=bv[:], in0=par_sb[:, :, 6 + L], in1=mscl[:], op=ALU.subtract
            )
            for jt in range(DJ):
                nc.vector.tensor_scalar(
                    h_out[:, jt, :],
                    h_out[:, jt, :],
                    scalar1=scl[:, jt : jt + 1],
                    scalar2=bv[:, jt : jt + 1],
                    op0=ALU.mult,
                    op1=ALU.add,
                )

        # L1: activation chunk ac = window (ranks interleave in h with stride
        # 2*WIN); depends only on gath[ac], so window 0 overlaps window-1 streaming
        def l1_rhs(kk, ac, sub):
            return gsbw[ac][:, 4 * sub : 4 * sub + 4, kk, :]

        def l1_out(h, jt, ac):
            return h[:, jt, :].rearrange("p (r w g) -> p r w g", w=WPC, g=WIN)[
                :, :, ac, :
            ]

        def mid_rhs(h_in):
            def rhs(kk, ac, sub):
                g0 = (2 * ac + sub) * 512
                return h_in[:, kk, g0 : g0 + 512]
            return rhs

        def mid_out(h, jt, ac):
            return h[:, jt, ac * (NG // 2) : (ac + 1) * (NG // 2)].rearrange(
                "p (a b) -> p a b", b=512
            )

        mlp_layer(0, w0_sb, K1, l1_rhs, l1_out, h0)
        mlp_layer(1, w1_sb, DJ, mid_rhs(h0), mid_out, h1)
        mlp_layer(2, w2_sb, DJ, mid_rhs(h1), mid_out, hout)

        for jt in range(DJ):
            nc.sync.dma_start(ap["out_t"][jt * P : (jt + 1) * P, :], hout[:, jt, :])


def _build_program(cfg):
    key = repr(sorted(cfg.items(), key=lambda kv: kv[0]))
    if key in _prog_cache:
        return _prog_cache[key]
    D, NG, EW, XW = cfg["D"], cfg["NG"], cfg["EW"], cfg["XW"]
    nt_e, nt_x = EW // P, XW // P
    nc = bacc.Bacc(
        "TRN2",
        target_bir_lowering=False,
        debug=False,
        enable_asserts=False,
        num_devices=NCORES,
    )
    ap = {}
    ins = [
        ("ea", [WPC * (nt_e // CH) * P, CH * D], F16),
        ("eg16", [P, WPC * nt_e], F16),
        ("ew16", [P, WPC * nt_e], F16),
        ("eg32", [P, WPC * nt_e], F32),
        ("ew32", [P, WPC * nt_e], F32),
        ("xa", [WPC * (nt_x // CH) * P, CH * D], F16),
        ("xg16", [P, WPC * nt_x], F16),
        ("xw16", [P, WPC * nt_x], F16),
        ("xg32", [P, WPC * nt_x], F32),
        ("xw32", [P, WPC * nt_x], F32),
        ("ut", [D, NG // NCORES], F16),
        ("w0t", [3 * D, D], F16),
        ("w1t", [D, D], F16),
        ("w2t", [D, D], F16),
        ("par", [D, 9], F32),
    ]
    for name, shape, dt in ins:
        ap[name] = nc.dram_tensor(name, shape, dt, kind="ExternalInput").ap()
    ap["out_t"] = nc.dram_tensor("out_t", [D, NG], F32, kind="ExternalOutput").ap()

    with tile.TileContext(nc) as tc:
        _emit(nc, tc, cfg, ap)
    nc.compile()
    _prog_cache[key] = nc
    return nc


# ---------------------------------------------------------------- host side


def _prepare(inputs):
    x = np.asarray(inputs["x"], dtype=np.float32)
    edge_attr = np.asarray(inputs["edge_attr"], dtype=np.float32)
    u = np.asarray(inputs["u"], dtype=np.float32)
    ei = np.asarray(inputs["edge_index"]).astype(np.int64)
    batch = np.asarray(inputs["batch"]).astype(np.int64)

    NN, D = x.shape
    NG = u.shape[0]
    WIN = NG // (NCORES * WPC)
    NWIN = NCORES * WPC

    src = ei[0]
    deg = np.bincount(src, minlength=NN).astype(np.float32)
    inv_deg = (1.0 / np.maximum(deg, 1.0)).astype(np.float32)
    cnt = np.bincount(batch, minlength=NG).astype(np.float32)
    inv_cnt = (1.0 / np.maximum(cnt, 1.0)).astype(np.float32)

    # nodes: sort by graph (setup_inputs already provides sorted batch)
    if np.any(batch[1:] < batch[:-1]):
        norder = np.argsort(batch, kind="stable")
        batch_s = batch[norder]
        x_s = x[norder]
    else:
        batch_s, x_s = batch, x

    gid = batch[src]
    w_e = inv_deg[src] * inv_cnt[gid]
    eorder = np.argsort(gid, kind="stable")
    gid_s = gid[eorder]
    w_e_s = w_e[eorder].astype(np.float32)
    w_n = inv_cnt[batch_s].astype(np.float32)

    wstarts = np.arange(NWIN + 1) * WIN
    e_bnd = np.searchsorted(gid_s, wstarts)
    x_bnd = np.searchsorted(batch_s, wstarts)
    EW = max(_ceil_to(int((e_bnd[1:] - e_bnd[:-1]).max()), CH * P), CH * P)
    XW = max(_ceil_to(int((x_bnd[1:] - x_bnd[:-1]).max()), CH * P), CH * P)
    nt_e, nt_x = EW // P, XW // P

    def tile_meta(sorted_gid, bnd, nt):
        # per program-window tile min/max local gid, unioned across cores
        lo_all = np.full((WPC, nt), np.inf)
        hi_all = np.full((WPC, nt), -np.inf)
        for c in range(NCORES):
            for wi in range(WPC):
                w = WPC * c + wi
                g = sorted_gid[bnd[w] : bnd[w + 1]] - w * WIN
                buf = np.full(nt * P, np.inf)
                buf[: len(g)] = g
                lo_all[wi] = np.minimum(lo_all[wi], buf.reshape(nt, P).min(1))
                buf = np.full(nt * P, -np.inf)
                buf[: len(g)] = g
                hi_all[wi] = np.maximum(hi_all[wi], buf.reshape(nt, P).max(1))
        meta = []
        for wi in range(WPC):
            bases = np.zeros(nt, np.int64)
            fixups = set()
            for t in range(1, nt):
                if not np.isfinite(lo_all[wi][t]):
                    bases[t] = 0
                    continue
                b = (int(lo_all[wi][t]) // SW) * SW
                bases[t] = b
                if b == 0 and int(hi_all[wi][t]) >= SW:
                    fixups.add(t)
            meta.append((tuple(bases.tolist()), tuple(sorted(fixups))))
        return meta

    emeta = tile_meta(gid_s, e_bnd, nt_e)
    xmeta = tile_meta(batch_s, x_bnd, nt_x)

    w0t = np.ascontiguousarray(np.asarray(inputs["W0"], np.float16).T)
    w1t = np.ascontiguousarray(np.asarray(inputs["W1"], np.float16).T)
    w2t = np.ascontiguousarray(np.asarray(inputs["W2"], np.float16).T)
    par = np.ascontiguousarray(
        np.stack(
            [np.asarray(inputs[k], np.float32) for k in
             ("b0", "b1", "b2", "g0", "g1", "g2", "be0", "be1", "be2")],
            axis=1,
        )
    )

    edge_attr_bf = edge_attr.astype(np.float16)[eorder]
    x_s_bf = x_s.astype(np.float16)

    def pack_core(c, data16, sorted_gid, wvals, bnd, nt, meta):
        """Chunk-major data + rebased gid/w tables for one core."""
        nch = nt // CH
        dat = np.zeros((WPC * nch * P, CH * D), np.float16)
        g32 = np.full((P, WPC * nt), -1.0, np.float32)
        w32 = np.zeros((P, WPC * nt), np.float32)
        for wi in range(WPC):
            w = WPC * c + wi
            lo, hi = int(bnd[w]), int(bnd[w + 1])
            n = hi - lo
            buf = np.zeros((nt * P, D), np.float16)
            buf[:n] = data16[lo:hi]
            dat[wi * nch * P : (wi + 1) * nch * P] = (
                buf.reshape(nch, CH, P, D).transpose(0, 2, 1, 3).reshape(nch * P, CH * D)
            )
            bases = np.asarray(meta[wi][0])
            gl = np.full(nt * P, -1.0, np.float32)
            gl[:n] = sorted_gid[lo:hi] - w * WIN
            gl = gl.reshape(nt, P)
            gl[1:] -= bases[1:, None]  # rebase (tile 0 keeps raw local gid)
            gl[gl < -1] = -1.0
            wv = np.zeros(nt * P, np.float32)
            wv[:n] = wvals[lo:hi]
            g32[:, wi * nt : (wi + 1) * nt] = gl.T
            w32[:, wi * nt : (wi + 1) * nt] = wv.reshape(nt, P).T
        return dat, g32, w32

    gpc = NG // NCORES
    in_maps = []
    for c in range(NCORES):
        ea_c, eg32, ew32 = pack_core(c, edge_attr_bf, gid_s, w_e_s, e_bnd, nt_e, emeta)
        xa_c, xg32, xw32 = pack_core(c, x_s_bf, batch_s, w_n, x_bnd, nt_x, xmeta)
        in_maps.append(
            {
                "ea": ea_c,
                "eg16": eg32.astype(np.float16), "ew16": ew32.astype(np.float16),
                "eg32": eg32, "ew32": ew32,
                "xa": xa_c,
                "xg16": xg32.astype(np.float16), "xw16": xw32.astype(np.float16),
                "xg32": xg32, "xw32": xw32,
                "ut": np.ascontiguousarray(u[c * gpc : (c + 1) * gpc].T.astype(np.float16)),
                "w0t": w0t, "w1t": w1t, "w2t": w2t, "par": par,
            }
        )

    cfg = {
        "D": D, "NG": NG, "EW": EW, "XW": XW,
        "emeta": tuple(emeta), "xmeta": tuple(xmeta),
    }
    return cfg, in_maps


def kernel(**inputs) -> np.ndarray:
    cfg, in_maps = _prepare(inputs)
    nc = _build_program(cfg)
    res = bass_utils.run_bass_kernel_spmd(nc, in_maps, core_ids=list(range(NCORES)))
    out_t = res.results[0]["out_t"]
    return np.ascontiguousarray(out_t.T)



# revision 17
# speedup vs baseline: 2.6209x; 2.6209x over previous
"""Trainium2 Bass kernel: MEGNet GlobalModel (graph aggregation + 3-layer MLP w/ BatchNorm).

Strategy (graph-parallel over 8 NeuronCores, fp8 streaming):
  - 2048 graphs -> core c owns graphs [256c, 256c+256). Host folds the two
    chained scatter_means into ONE weighted segment-sum per stream:
        u_e[g] = sum_{e: batch[src_e]=g} edge_attr[e] * inv_deg[src_e] * inv_cnt[g]
        u_v[g] = sum_{n: batch[n]=g}    x[n] * inv_cnt[g]
    The per-row weight is folded into the DATA on host, scaled by S (a power of
    two, un-done inside W0), and quantized to fp8 e4m3. Device reduction is a
    sorted segment-sum via one-hot eq-matrices multiplied on the PE.
  - Rows are sorted by graph and padded so every 256-row double-tile lies
    within ONE 32-graph slot (padding schedule shared across cores: max over
    cores per 32-graph group). fp8 DoubleRow matmuls contract 256 rows per
    instruction at 0.5 cyc/row: acc[slot32, 256f] += eq[128,2,32].T x data[128,2,256].
  - eq one-hot masks are built on DVE per 16-double-tile chunk from a tiny
    per-row gid table (fp16 compare, fp8 out).
  - MLP is SHARDED: each core runs the 3-layer MLP on its own 256 graphs in
    [feature, graph] layout; BatchNorm batch stats are computed as local
    partial sums and combined with three 2KB AllReduces. Each core writes only
    its 256-graph output shard; the host assembles the full [2048, 256] output.
"""

import sys

sys.path.insert(0, "/opt/trn_rl_repo")

import ml_dtypes
import numpy as np

from concourse import bacc, bass, bass_utils, mybir, tile
from concourse.masks import make_identity

F32 = mybir.dt.float32
F16 = mybir.dt.float16
F8 = mybir.dt.float8e4
P = 128
EPS = 1e-5
NCORES = 8
CH = 16        # double-tiles per DMA chunk (4096 rows, 8KB/partition fp8)
SW = 64        # graph slot width of a double-tile
NGRP = 4       # 64-graph groups per core (G=256)
ALU = mybir.AluOpType
ACTF = mybir.ActivationFunctionType
DR = mybir.MatmulPerfMode.DoubleRow
E4M3 = ml_dtypes.float8_e4m3

_prog_cache: dict = {}

# fp16->e4m3 and f32-top16->e4m3 LUTs (saturating, nan->0)
def _mk_lut():
    u = np.arange(65536, dtype=np.uint32)
    vals = ((u << np.uint32(16)) | np.uint32(0x8000)).view(np.float32)
    vals = np.nan_to_num(vals, nan=0.0, posinf=240.0, neginf=-240.0)
    vals = np.clip(vals, -240.0, 240.0)
    with np.errstate(invalid="ignore"):
        return vals.astype(E4M3).view(np.uint8)


_LUT32 = _mk_lut()


def _to_fp8(y32):
    """f32 array -> e4m3 bytes via top-16-bit LUT (round-to-nearest-ish)."""
    hi = y32.view(np.uint16)[..., 1::2]
    return np.take(_LUT32, hi)


def _ceil_to(a: int, m: int) -> int:
    return -(-a // m) * m


# ---------------------------------------------------------------- device program


def _emit(nc, tc, cfg, ap):
    D, G = cfg["D"], cfg["G"]
    NG = G * NCORES
    DJ = D // P           # feature tiles (2)
    K1 = 3 * D // P       # k-tiles of layer 1 (6)
    NT2E, NT2X = cfg["NT2E"], cfg["NT2X"]
    esched, xsched = cfg["esched"], cfg["xsched"]
    assert G == 2 * P and D == 2 * P

    def flags(sched):
        # per double-tile: (feature col offset, start, stop)
        # group g of 64 graphs -> acc partitions 0:64, col block g*D
        # (DoubleRow matmuls may only write PSUM partition offset 0)
        out = []
        for g, n in enumerate(sched):
            for i in range(n):
                out.append((g * D, i == 0, i == n - 1))
        return out

    eflags, xflags = flags(esched), flags(xsched)
    assert len(eflags) == NT2E and len(xflags) == NT2X

    with (
        tc.tile_pool(name="const", bufs=1) as cpool,
        tc.tile_pool(name="tables", bufs=1) as tpool,
        tc.tile_pool(name="data", bufs=4) as dpool,
        tc.tile_pool(name="eq", bufs=3) as qpool,
        tc.tile_pool(name="evac", bufs=2) as epool,
        tc.tile_pool(name="acc", bufs=1, space="PSUM") as apool,
        tc.tile_pool(name="psum", bufs=2, space="PSUM") as ppool,
        tc.tile_pool(name="mlp", bufs=1) as mpool,
        tc.tile_pool(name="stats", bufs=2) as stpool,
        tc.tile_pool(name="dram", bufs=1, space="DRAM") as drpool,
    ):
        # --- constants
        ident = cpool.tile([P, P], F32)
        make_identity(nc, ident[:])
        iota_i = cpool.tile([P, SW], mybir.dt.int32)
        nc.gpsimd.iota(iota_i[:], pattern=[[1, SW]], base=0, channel_multiplier=0)
        iota16 = cpool.tile([P, SW], F16)
        nc.vector.tensor_copy(iota16[:], iota_i[:])
        iota3 = iota16[:, 0:SW].rearrange("p (o f) -> p o f", o=1)
        eps_sb = cpool.tile([P, 1], F32)
        nc.vector.memset(eps_sb[:], EPS)

        # --- per-row gid tables (rebased to slot base, pad rows = -1)
        eg = tpool.tile([P, NT2E, 2], F16)
        nc.scalar.dma_start(eg[:], ap["eg"][:, :, :])
        xg = tpool.tile([P, NT2X, 2], F16)
        nc.scalar.dma_start(xg[:], ap["xg"][:, :, :])

        # --- MLP params + u block of combT
        combT = mpool.tile([P, K1, G], F16, name="combT")
        nc.scalar.dma_start(
            combT[:, 2 * DJ : 3 * DJ, :],
            ap["utT"][:, :].rearrange("(a p) g -> p a g", p=P),
        )
        w0_sb = mpool.tile([P, K1, D], F16, name="w0")
        nc.scalar.dma_start(w0_sb[:], ap["w0t"][:, :].rearrange("(a p) f -> p a f", p=P))
        w1_sb = mpool.tile([P, DJ, D], F16, name="w1")
        nc.scalar.dma_start(w1_sb[:], ap["w1t"][:, :].rearrange("(a p) f -> p a f", p=P))
        w2_sb = mpool.tile([P, DJ, D], F16, name="w2")
        nc.scalar.dma_start(w2_sb[:], ap["w2t"][:, :].rearrange("(a p) f -> p a f", p=P))
        par_sb = mpool.tile([P, DJ, 9], F32, name="par")
        nc.scalar.dma_start(par_sb[:], ap["par"][:, :].rearrange("(a p) c -> p a c", p=P))

        # --- DRAM bounce buffers for BN stat AllReduces
        st_in = [drpool.tile([P, 2 * DJ], F32, name=f"sti{i}") for i in range(3)]
        st_out = [
            drpool.tile([P, 2 * DJ], F32, addr_space="Shared", name=f"sto{i}")
            for i in range(3)
        ]

        # --- weighted segment-sum of one fp8 stream into acc [slot128, 2*D]
        def seg_stream(data_ap, g_sb, nt2, fl, acc):
            nchunks = nt2 // CH
            for c in range(nchunks):
                chunk = dpool.tile([P, CH, 2, D], F8, tag="data")
                eng = nc.sync if c % 2 == 0 else nc.scalar
                eng.dma_start(chunk[:], data_ap[c * P : (c + 1) * P, :])
                eq = qpool.tile([P, CH, 2, SW], F8, tag="eq")
                nc.vector.tensor_tensor(
                    out=eq[:].rearrange("p c h j -> p (c h) j"),
                    in0=iota3.to_broadcast([P, CH * 2, SW]),
                    in1=g_sb[:, c * CH : (c + 1) * CH, :]
                    .rearrange("p c (h o) -> p (c h) o", o=1)
                    .to_broadcast([P, CH * 2, SW]),
                    op=ALU.is_equal,
                )
                for t in range(CH):
                    fo, st, sp = fl[c * CH + t]
                    nc.tensor.matmul(
                        acc[0:SW, fo : fo + D],
                        lhsT=eq[:, t, :, :],
                        rhs=chunk[:, t, :, :],
                        start=st,
                        stop=sp,
                        perf_mode=DR,
                        skip_group_check=True,
                    )

        ab = cfg.get("ab", ())
        acc_e = apool.tile([P, NGRP * D], F32, tag="acce")
        acc_x = apool.tile([P, NGRP * D], F32, tag="accx")
        seg_stream(ap["ea"], eg, NT2E, eflags, acc_e)
        evacs = [(0, acc_e)]
        if "nox" not in ab:
            seg_stream(ap["xa"], xg, NT2X, xflags, acc_x)
            evacs.append((DJ, acc_x))

        # --- evacuate acc -> combT (transpose [graph, feat] -> [feat, graph])
        # col block g holds graphs 64g..64g+63 on partitions 0:64
        for k0, acc in evacs:
            for g in range(NGRP):
                for kf in range(DJ):
                    asb = epool.tile([P, P], F32, tag="asb")
                    nc.scalar.copy(
                        asb[0:SW, :], acc[0:SW, g * D + kf * P : g * D + (kf + 1) * P]
                    )
                    tp = ppool.tile([P, SW], F32, tag="tp")
                    nc.tensor.transpose(tp[:], asb[0:SW, :], ident[0:SW, 0:SW])
                    nc.scalar.copy(combT[:, k0 + kf, g * SW : (g + 1) * SW], tp[:])

        # --- sharded 3-layer MLP on this core's G graphs, [feature, graph] layout
        h0 = mpool.tile([P, DJ, G], F16, name="h0")
        h1 = mpool.tile([P, DJ, G], F16, name="h1")
        hout = mpool.tile([P, DJ, G], F32, name="hout")

        def mlp_layer(L, w_sb, nk, rhs_of, h_out):
            st_sb = stpool.tile([P, DJ, 2], F32, tag="stsb")
            for jt in range(DJ):
                mm = ppool.tile([P, G], F32, tag="mm")
                for k in range(nk):
                    nc.tensor.matmul(
                        mm[:],
                        lhsT=w_sb[:, k, jt * P : (jt + 1) * P],
                        rhs=rhs_of(k),
                        start=(k == 0),
                        stop=(k == nk - 1),
                    )
                hsl = h_out[:, jt, :]
                nc.scalar.activation(
                    hsl,
                    mm[:],
                    ACTF.Relu,
                    bias=par_sb[:, jt, L : L + 1],
                    scale=1.0,
                    accum_out=st_sb[:, jt, 0:1],
                )
                scr = stpool.tile([P, G], F16, tag="scr")
                nc.scalar.activation(
                    scr[:], hsl, ACTF.Square, scale=1.0,
                    accum_out=st_sb[:, jt, 1:2],
                )
            # AllReduce the per-core (sum, sumsq) partials
            if "nocc" not in ab:
                nc.sync.dma_start(
                    st_in[L][:, :], st_sb[:].rearrange("p a b -> p (a b)")
                )
                nc.gpsimd.collective_compute(
                    "AllReduce",
                    ALU.add,
                    replica_groups=[list(range(NCORES))],
                    ins=[st_in[L].opt()],
                    outs=[st_out[L].opt()],
                )
                r_st = stpool.tile([P, DJ, 2], F32, tag="rst")
                nc.scalar.dma_start(
                    r_st[:], st_out[L][:, :].rearrange("p (a b) -> p a b", a=DJ)
                )
            else:
                r_st = st_sb
            # BatchNorm finalize (global batch = NG graphs)
            mean = stpool.tile([P, DJ], F32, tag="mean")
            nc.vector.tensor_scalar_mul(mean[:], r_st[:, :, 0], 1.0 / NG)
            esq = stpool.tile([P, DJ], F32, tag="esq")
            nc.vector.tensor_scalar_mul(esq[:], r_st[:, :, 1], 1.0 / NG)
            m2 = stpool.tile([P, DJ], F32, tag="m2")
            nc.vector.tensor_tensor(out=m2[:], in0=mean[:], in1=mean[:], op=ALU.mult)
            var = stpool.tile([P, DJ], F32, tag="var")
            nc.vector.tensor_tensor(out=var[:], in0=esq[:], in1=m2[:], op=ALU.subtract)
            std = stpool.tile([P, DJ], F32, tag="std")
            nc.scalar.activation(std[:], var[:], ACTF.Sqrt, bias=eps_sb[:], scale=1.0)
            rstd = stpool.tile([P, DJ], F32, tag="rstd")
            nc.vector.reciprocal(rstd[:], std[:])
            scl = stpool.tile([P, DJ], F32, tag="scl")
            nc.vector.tensor_tensor(
                out=scl[:], in0=rstd[:], in1=par_sb[:, :, 3 + L], op=ALU.mult
            )
            mscl = stpool.tile([P, DJ], F32, tag="mscl")
            nc.vector.tensor_tensor(out=mscl[:], in0=mean[:], in1=scl[:], op=ALU.mult)
            bv = stpool.tile([P, DJ], F32, tag="bv")
            nc.vector.tensor_tensor(
                out=bv[:], in0=par_sb[:, :, 6 + L], in1=mscl[:], op=ALU.subtract
            )
            for jt in range(DJ):
                nc.vector.tensor_scalar(
                    h_out[:, jt, :],
                    h_out[:, jt, :],
                    scalar1=scl[:, jt : jt + 1],
                    scalar2=bv[:, jt : jt + 1],
                    op0=ALU.mult,
                    op1=ALU.add,
                )

        if "dbg" in ap:
            nc.sync.dma_start(
                ap["dbg"][:, :].rearrange("(a p) g -> p a g", p=P), combT[:]
            )

        if "nomlp" not in ab:
            mlp_layer(0, w0_sb, K1, lambda k: combT[:, k, :], h0)
            mlp_layer(1, w1_sb, DJ, lambda k: h0[:, k, :], h1)
            mlp_layer(2, w2_sb, DJ, lambda k: h1[:, k, :], hout)
        else:
            # crash-bisect mode: copy evac result so out depends on the streams
            for jt in range(DJ):
                nc.vector.tensor_copy(hout[:, jt, :], combT[:, jt, :])

        nc.sync.dma_start(
            ap["out_t"][:, :].rearrange("(a p) g -> p a g", p=P), hout[:]
        )


def _build_program(cfg):
    key = repr(sorted(cfg.items(), key=lambda kv: kv[0]))
    if key in _prog_cache:
        return _prog_cache[key]
    D, G = cfg["D"], cfg["G"]
    NT2E, NT2X = cfg["NT2E"], cfg["NT2X"]
    nc = bacc.Bacc(
        "TRN2",
        target_bir_lowering=False,
        debug=False,
        enable_asserts=False,
        num_devices=NCORES,
    )
    ap = {}
    ins = [
        ("ea", [(NT2E // CH) * P, CH * 2 * D], F8),
        ("eg", [P, NT2E, 2], F16),
        ("xa", [(NT2X // CH) * P, CH * 2 * D], F8),
        ("xg", [P, NT2X, 2], F16),
        ("utT", [D, G], F16),
        ("w0t", [3 * D, D], F16),
        ("w1t", [D, D], F16),
        ("w2t", [D, D], F16),
        ("par", [D, 9], F32),
    ]
    for name, shape, dt in ins:
        ap[name] = nc.dram_tensor(name, shape, dt, kind="ExternalInput").ap()
    ap["out_t"] = nc.dram_tensor("out_t", [D, G], F32, kind="ExternalOutput").ap()
    if cfg.get("dbg"):
        ap["dbg"] = nc.dram_tensor("dbg", [3 * D, G], F16, kind="ExternalOutput").ap()

    with tile.TileContext(nc) as tc:
        _emit(nc, tc, cfg, ap)
    nc.compile()
    _prog_cache[key] = nc
    return nc


# ---------------------------------------------------------------- host side


def _pack_stream(data, w, order, sorted_gid, NG):
    """Sort+pad+quantize one stream for all cores.

    Returns (per-core packed fp8 [nch*P, CH*2*D], per-core gid tables
    [P, NT2, 2] f16, shared schedule tuple, NT2, scale S)."""
    N, D = data.shape
    # power-of-two scale keeping all quantized values inside e4m3 range
    M = float((np.abs(data).max(axis=1) * w).max())
    S = float(2.0 ** np.clip(np.floor(np.log2(224.0 / max(M, 1e-30))), -24, 24))
    G = NG // NCORES
    gs = sorted_gid  # [N] sorted
    # group = 32-graph bucket, global id in [0, NCORES*NGRP)
    grp = gs // SW
    gcnt = np.bincount(grp, minlength=NCORES * NGRP).reshape(NCORES, NGRP)
    # shared schedule: tiles per group = max over cores, >= 1
    sched = np.maximum((-(-gcnt // 256)).max(axis=0), 1)
    nt2 = int(sched.sum())
    NT2 = max(_ceil_to(nt2, CH), CH)
    sched[NGRP - 1] += NT2 - nt2
    t0s = np.concatenate([[0], np.cumsum(sched)])  # tile starts per group

    # quantize (weight+scale folded in)
    y = np.multiply(data, (w * S)[:, None])
    q8 = np.empty((N + 1, D), np.uint8)
    hi = y.view(np.uint16)[:, 1::2]
    np.take(_LUT32, hi, out=q8[:N])
    q8[N] = 0  # sentinel zero row

    bnd = np.searchsorted(grp, np.arange(NCORES * NGRP + 1))
    nch = NT2 // CH
    packs, gtabs = [], []
    for c in range(NCORES):
        srcrow = np.full(NT2 * 256, N, np.int64)
        grb = np.full(NT2 * 256, -1.0, np.float32)
        for g in range(NGRP):
            lo, hi_ = int(bnd[c * NGRP + g]), int(bnd[c * NGRP + g + 1])
            n = hi_ - lo
            o = int(t0s[g]) * 256
            srcrow[o : o + n] = order[lo:hi_]
            grb[o : o + n] = gs[lo:hi_] - (c * G + SW * g)
        idx = srcrow.reshape(nch, CH, 2, P).transpose(0, 3, 1, 2).reshape(nch * P, CH * 2)
        packs.append(q8[idx].reshape(nch * P, CH * 2 * D).view(E4M3))
        gtabs.append(
            np.ascontiguousarray(
                grb.reshape(NT2, 2, P).transpose(2, 0, 1).astype(np.float16)
            )
        )
    return packs, gtabs, tuple(int(s) for s in sched), NT2, S


def _prepare(inputs):
    x = np.asarray(inputs["x"], dtype=np.float32)
    edge_attr = np.asarray(inputs["edge_attr"], dtype=np.float32)
    u = np.asarray(inputs["u"], dtype=np.float32)
    ei = np.asarray(inputs["edge_index"]).astype(np.int64)
    batch = np.asarray(inputs["batch"]).astype(np.int64)

    NN, D = x.shape
    NG = u.shape[0]
    G = NG // NCORES

    src = ei[0]
    deg = np.bincount(src, minlength=NN).astype(np.float32)
    inv_deg = (1.0 / np.maximum(deg, 1.0)).astype(np.float32)
    cnt = np.bincount(batch, minlength=NG).astype(np.float32)
    inv_cnt = (1.0 / np.maximum(cnt, 1.0)).astype(np.float32)

    if np.any(batch[1:] < batch[:-1]):
        norder = np.argsort(batch, kind="stable")
    else:
        norder = np.arange(NN)
    batch_s = batch[norder]
    w_n = inv_cnt[batch]  # original node order

    gid = batch[src]
    eorder = np.argsort(gid, kind="stable")
    gid_s = gid[eorder]
    w_e = (inv_deg[src] * inv_cnt[gid]).astype(np.float32)  # original edge order

    epacks, egtabs, esched, NT2E, s_e = _pack_stream(edge_attr, w_e, eorder, gid_s, NG)
    xpacks, xgtabs, xsched, NT2X, s_x = _pack_stream(x, w_n, norder, batch_s, NG)

    w0 = np.asarray(inputs["W0"], np.float32)  # [D, 3D]
    w0t = w0.T.copy()
    w0t[0:D] /= s_e
    w0t[D : 2 * D] /= s_x
    w0t = w0t.astype(np.float16)
    w1t = np.ascontiguousarray(np.asarray(inputs["W1"], np.float16).T)
    w2t = np.ascontiguousarray(np.asarray(inputs["W2"], np.float16).T)
    par = np.ascontiguousarray(
        np.stack(
            [np.asarray(inputs[k], np.float32) for k in
             ("b0", "b1", "b2", "g0", "g1", "g2", "be0", "be1", "be2")],
            axis=1,
        )
    )

    in_maps = []
    for c in range(NCORES):
        in_maps.append(
            {
                "ea": epacks[c], "eg": egtabs[c],
                "xa": xpacks[c], "xg": xgtabs[c],
                "utT": np.ascontiguousarray(u[c * G : (c + 1) * G].T.astype(np.float16)),
                "w0t": w0t, "w1t": w1t, "w2t": w2t, "par": par,
            }
        )

    cfg = {
        "D": D, "G": G, "NT2E": NT2E, "NT2X": NT2X,
        "esched": esched, "xsched": xsched,
    }
    return cfg, in_maps


def kernel(**inputs) -> np.ndarray:
    cfg, in_maps = _prepare(inputs)
    nc = _build_program(cfg)
    res = bass_utils.run_bass_kernel_spmd(nc, in_maps, core_ids=list(range(NCORES)))
    G = cfg["G"]
    out = np.empty((G * NCORES, cfg["D"]), np.float32)
    for c in range(NCORES):
        out[c * G : (c + 1) * G] = res.results[c]["out_t"].T
    return out
